# revision 10
# baseline (speedup 1.0000x reference)
"""Causal single-head attention (B=4, T=4096, C=1024, H=64) on 8 TRN2 NeuronCores.

Sharding: 2 cores per batch element. Core s of a pair owns q blocks
{s, 2+s, 5-s, 7-s} (512 rows each) -> 18 causal kv-units per core (balanced).
Each owned block is a "slot" with a uniform kv-tile count {8,16,24,32} across
both cores; 8 surplus tiles per core are zero-masked dummies so the SPMD
stream is identical and only mask/Q-projection addresses are pid-affine.

Performance structure (v2):
  - x arrives host-packed as f16 [128, 8, 4096] (partition-major c-chunks):
    halves HBM traffic, no on-chip casts, and each 2 MiB piece is ONE DMA
    instruction (2 KB descriptor lines) so the SP queue isn't issue-bound.
  - Piece order {6,7},{0,1},{2,3},{4,5}: the last q block's Q projection and
    kv tiles 24-31 are computed FIRST, so slot3 (32 tiles) accumulates its
    tail tiles early (PV accumulation order is commutative) and the
    post-stream drain is only the last ~32 interleaved tiles.
  - Causal masks are built on the idle GpSimd engine (memset+affine_select),
    not DMA'd; mask select for diag/ones/zeros is pid-affine into one table.
  - exp runs on ACT only, fused over kv-tile pairs ([128,1024] PSUM reads);
    PSUM->SBUF copies on DVE; PE does only matmuls + V transposes.
  - KV projections for two t-blocks are interleaved c-outer so the stationary
    weight tile is loaded once per pair of matmuls.
  - The softmax division and final [H,q]->[q,H] transpose happen on the HOST;
    the kernel emits O^T with the row-sum appended ([65, 512] f32 per slot).
  - Dummy ident matmuls pad the PE stream while the first x piece streams in
    so the HAM clock gate reaches 8/8 (2.4 GHz) early and stays there.
"""

import numpy as np

import concourse.bacc as bacc
import concourse.bass as bass
import concourse.mybir as mybir
import concourse.tile as tile
from concourse.bass_utils import run_bass_kernel_spmd
from concourse.masks import make_identity

B, T, C, H = 4, 4096, 1024, 64
NCORES = 8
TB = 512                  # q/t block width
NKVT = T // 128           # 32 kv tiles of 128
SLOT_TILES = [8, 16, 24, 32]
F32 = mybir.dt.float32
F16 = mybir.dt.float16

_nc = None


def _build():
    nc = bacc.Bacc("TRN2", target_bir_lowering=False, debug=False, num_devices=NCORES)
    xt = nc.dram_tensor("xt", [128, 8, T], F16, kind="ExternalInput").ap()
    wq = nc.dram_tensor("wq", [128, 8 * H], F16, kind="ExternalInput").ap()
    wkv = nc.dram_tensor("wkv", [128, 8 * 2 * H], F16, kind="ExternalInput").ap()
    out = nc.dram_tensor("out", [H + 1, 4 * TB], F32, kind="ExternalOutput").ap()

    PEX = mybir.EngineType.PE
    DVE = mybir.EngineType.DVE

    with tile.TileContext(nc) as tc:
        pid = nc.partition_id(engines=[PEX, DVE])
        s = pid % 2
        sn = (pid + 1) % 2
        with tc.tile_pool(name="persist", bufs=1) as persist, \
             tc.tile_pool(name="work", bufs=1) as work, \
             tc.tile_pool(name="pp", bufs=1, space="PSUM") as pp:
            ident = persist.tile([128, 128], F16)
            make_identity(nc, ident)
            junk = persist.tile([128, TB], F16)   # warmup moving operand
            nc.gpsimd.memset(junk, 0.0)
            wq_sb = persist.tile([128, 8 * H], F16)
            wkv_sb = persist.tile([128, 8 * 2 * H], F16)
            nc.scalar.dma_start(out=wq_sb, in_=wq)
            nc.scalar.dma_start(out=wkv_sb, in_=wkv)

            # masks: [zeros, d0, d1, d2, d3, ones] built on GpSimd
            masks = persist.tile([128, 6 * TB], F16)
            nc.gpsimd.memset(masks[:, TB:], 1.0)
            nc.gpsimd.memset(masks[:, 0:TB], 0.0)
            for j in range(4):
                nc.gpsimd.affine_select(
                    out=masks[:, (1 + j) * TB:(2 + j) * TB],
                    in_=masks[:, (1 + j) * TB:(2 + j) * TB],
                    pattern=[[1, TB]],
                    compare_op=mybir.AluOpType.is_ge,
                    fill=0.0,
                    base=-128 * j,
                    channel_multiplier=-1,
                )

            xbig = persist.tile([128, 8, T], F16)
            QT = persist.tile([64, 4 * TB], F16)     # slot-compact Q^T
            KT = persist.tile([64, T], F16)
            V = persist.tile([128, NKVT, H + 1], F16)
            nc.gpsimd.memset(V[:, :, H], 1.0)        # row-sum ones column

            # x pieces (2 t-blocks each), stream order {6,7},{0,1},{2,3},{4,5}
            for p0 in (3072, 0, 1024, 2048):
                nc.sync.dma_start(
                    out=xbig[:, :, p0:p0 + 1024], in_=xt[:, :, p0:p0 + 1024]
                )

            def warm(n):
                for _ in range(n):
                    pw = pp.tile([128, 2 * TB], F32, name="pw", tag="ps", bufs=2)
                    nc.tensor.matmul(pw[:, 0:TB], ident, junk, start=True,
                                     stop=True)

            def emit_qproj(slot, off):
                psq = pp.tile([128, TB], F32, name="psq", tag="pj", bufs=2)
                for c in range(8):
                    nc.tensor.matmul(
                        psq[0:64, :],
                        wq_sb[:, c * H:(c + 1) * H],
                        xbig[:, c, bass.ds(off, TB)],
                        start=(c == 0),
                        stop=(c == 7),
                    )
                nc.vector.tensor_copy(QT[:, slot * TB:(slot + 1) * TB], psq[0:64, :])

            def emit_kvproj_pair(tb0):
                tbs = (tb0, tb0 + 1)
                pv = []
                for tb in tbs:
                    pv.append(pp.tile([128, TB], F32, name=f"pvk{tb}", tag="pj",
                                      bufs=2))
                for c in range(8):
                    for i, tb in enumerate(tbs):
                        nc.tensor.matmul(
                            pv[i],
                            wkv_sb[:, c * 128:(c + 1) * 128],
                            xbig[:, c, tb * TB:(tb + 1) * TB],
                            start=(c == 0),
                            stop=(c == 7),
                        )
                for i, tb in enumerate(tbs):
                    nc.vector.tensor_copy(KT[:, tb * TB:(tb + 1) * TB], pv[i][0:64, :])
                    vt = work.tile([64, TB], F16, name="vt", tag="vt", bufs=2)
                    nc.vector.tensor_copy(vt, pv[i][64:128, :])
                    psv = pp.tile([128, TB], F16, name="psv", tag="pj", bufs=2)
                    for j in range(4):
                        nc.tensor.transpose(
                            psv[:, j * 64:(j + 1) * 64],
                            vt[:, j * 128:(j + 1) * 128],
                            ident[0:64, 0:64],
                        )
                    nc.vector.tensor_copy(V[:, 4 * tb:4 * tb + 4, 0:H], psv[:, 0:256])

            def mask_off(slot, r):
                # table [zeros,d0..d3,ones]; offsets affine in s, nonneg coeffs
                if slot < 2:
                    if r < 4:   # s=0: diag r, s=1: ones
                        return TB * (1 + r) + s * (TB * (4 - r))
                    else:       # s=0: zeros, s=1: diag r-4
                        return s * (TB * (r - 3))
                else:
                    assert r >= 4   # s=0: diag r-4 (TB*(r-3)), s=1: zeros (0)
                    return sn * (TB * (r - 3))

            def mask_off23_low(r):
                # slots 2,3 / r<4: s=0 -> ones (5*TB), s=1 -> diag r ((1+r)*TB)
                return nc.s_assert_within(
                    sn * (TB * 5) + s * (TB * (1 + r)), None, TB * 5,
                    skip_runtime_assert=True,
                )

            st = {}

            def emit_attn(slot, ks):
                cnt = SLOT_TILES[slot]
                first = (ks[0] == (24 if slot == 3 else 0))
                last_k = 23 if slot == 3 else cnt - 1
                if first:
                    st[slot] = {
                        "po": pp.tile([H + 1, TB], F32, name=f"po{slot}",
                                      tag=("po_b" if slot == 3 else "po_a"),
                                      bufs=1),
                        "first": True,
                    }
                sd = st[slot]
                for idx in range(0, len(ks), 2):
                    kpair = ks[idx:idx + 2]
                    ps = pp.tile([128, 2 * TB], F32, name="ps", tag="ps", bufs=2)
                    for h, k in enumerate(kpair):
                        nc.tensor.matmul(
                            ps[:, h * TB:(h + 1) * TB],
                            KT[:, k * 128:(k + 1) * 128],
                            QT[:, slot * TB:(slot + 1) * TB],
                            start=True,
                            stop=True,
                        )
                    pt = work.tile([128, 2 * TB], F16, name="pt", tag="pt", bufs=4)
                    nc.scalar.activation(
                        pt, ps, mybir.ActivationFunctionType.Exp, scale=0.125
                    )
                    for h, k in enumerate(kpair):
                        r = k - (cnt - 8)
                        if r >= 0:
                            if slot >= 2 and r < 4:
                                off = mask_off23_low(r)
                            else:
                                off = mask_off(slot, r)
                            nc.vector.tensor_mul(
                                pt[:, h * TB:(h + 1) * TB],
                                pt[:, h * TB:(h + 1) * TB],
                                masks[:, bass.ds(off, TB)],
                            )
                    for h, k in enumerate(kpair):
                        nc.tensor.matmul(
                            sd["po"],
                            V[:, k, :],
                            pt[:, h * TB:(h + 1) * TB],
                            start=sd["first"],
                            stop=(k == last_k),
                        )
                        sd["first"] = False
                if ks[-1] == last_k:
                    ot = work.tile([H + 1, TB], F32, name="ot", tag="ot", bufs=2)
                    nc.vector.tensor_copy(ot, sd["po"])
                    nc.sync.dma_start(
                        out=out[:, slot * TB:(slot + 1) * TB], in_=ot
                    )

            # ---- emission schedule ----
            warm(30)
            emit_qproj(3, 3072 + sn * TB)      # block 7-s from piece {6,7}
            emit_kvproj_pair(6)                # kv tiles 24-31
            emit_attn(3, list(range(24, 32)))  # slot3 tail tiles first
            emit_kvproj_pair(0)
            emit_qproj(0, s * TB)              # block s
            emit_attn(0, list(range(0, 8)))
            emit_attn(3, list(range(0, 8)))
            emit_kvproj_pair(2)
            emit_qproj(1, 1024 + s * TB)       # block 2+s
            emit_attn(1, list(range(0, 16)))
            emit_attn(3, list(range(8, 16)))
            emit_kvproj_pair(4)
            emit_qproj(2, 2048 + sn * TB)      # block 5-s
            emit_attn(2, list(range(0, 24)))
            emit_attn(3, list(range(16, 24)))

    nc.compile()
    return nc


def get_nc():
    global _nc
    if _nc is None:
        _nc = _build()
    return _nc


def _pack_w(wt):
    # [C, M] (= W.T) -> [128, 8*M]: partition p, free c*M+m = wt[c*128+p, m]
    M = wt.shape[1]
    return np.ascontiguousarray(
        wt.reshape(8, 128, M).transpose(1, 0, 2).reshape(128, 8 * M)
    )


def make_inputs(x, Wq, Wk, Wv):
    x = np.asarray(x, dtype=np.float32).astype(np.float16)
    wq_in = _pack_w(np.asarray(Wq, np.float32).T.astype(np.float16))
    wkv_in = _pack_w(
        np.concatenate(
            [np.asarray(Wk, np.float32).T, np.asarray(Wv, np.float32).T], axis=1
        ).astype(np.float16)
    )
    in_maps = []
    for core in range(NCORES):
        b = core // 2
        # [T, C] -> [C, T] -> [8, 128, T] -> [128, 8, T]
        xp = np.ascontiguousarray(
            x[b].T.reshape(8, 128, T).transpose(1, 0, 2)
        )
        in_maps.append({"xt": xp, "wq": wq_in, "wkv": wkv_in})
    return in_maps


def gather_output(results):
    """results: per-core {"out": [65, 2048] f32} -> full [B, T, H] f32."""
    O = np.empty((B, T, H), np.float32)
    for core in range(NCORES):
        b, sv = core // 2, core % 2
        o = results[core]["out"].astype(np.float64)
        blocks = [sv, 2 + sv, 5 - sv, 7 - sv]
        for slot, g in enumerate(blocks):
            ot = o[0:64, slot * TB:(slot + 1) * TB]
            l = o[64, slot * TB:(slot + 1) * TB]
            O[b, g * TB:(g + 1) * TB] = (ot / l).T.astype(np.float32)
    return O


def kernel(x, Wq, Wk, Wv):
    nc = get_nc()
    in_maps = make_inputs(x, Wq, Wk, Wv)
    res = run_bass_kernel_spmd(nc, in_maps, list(range(NCORES)))
    return gather_output(res.results)


# revision 15
# speedup vs baseline: 1.0063x; 1.0063x over previous
"""Causal single-head attention (B=4, T=4096, C=1024, H=64) on 8 TRN2 NeuronCores.

Sharding: 2 cores per batch element. Core s of a pair owns q blocks
{s, 2+s, 5-s, 7-s} (512 rows each) -> 18 causal kv-units per core (balanced).
Each owned block is a "slot" with a uniform kv-tile count {8,16,24,32} across
both cores; 8 surplus tiles per core are zero-masked dummies so the SPMD
stream is identical and only mask/Q-projection addresses are pid-affine.

Performance structure (v2):
  - x arrives host-packed as f16 [128, 8, 4096] (partition-major c-chunks):
    halves HBM traffic, no on-chip casts, and each 2 MiB piece is ONE DMA
    instruction (2 KB descriptor lines) so the SP queue isn't issue-bound.
  - Piece order {6,7},{0,1},{2,3},{4,5}: the last q block's Q projection and
    kv tiles 24-31 are computed FIRST, so slot3 (32 tiles) accumulates its
    tail tiles early (PV accumulation order is commutative) and the
    post-stream drain is only the last ~32 interleaved tiles.
  - Causal masks are built on the idle GpSimd engine (memset+affine_select),
    not DMA'd; mask select for diag/ones/zeros is pid-affine into one table.
  - exp runs on ACT only, fused over kv-tile pairs ([128,1024] PSUM reads);
    PSUM->SBUF copies on DVE; PE does only matmuls + V transposes.
  - KV projections for two t-blocks are interleaved c-outer so the stationary
    weight tile is loaded once per pair of matmuls.
  - The softmax division and final [H,q]->[q,H] transpose happen on the HOST;
    the kernel emits O^T with the row-sum appended ([65, 512] f32 per slot).
  - Dummy ident matmuls pad the PE stream while the first x piece streams in
    so the HAM clock gate reaches 8/8 (2.4 GHz) early and stays there.
"""

import numpy as np

import concourse.bacc as bacc
import concourse.bass as bass
import concourse.mybir as mybir
import concourse.tile as tile
from concourse.bass_utils import run_bass_kernel_spmd
from concourse.masks import make_identity

B, T, C, H = 4, 4096, 1024, 64
NCORES = 8
TB = 512                  # q/t block width
NKVT = T // 128           # 32 kv tiles of 128
SLOT_TILES = [8, 16, 24, 32]
F32 = mybir.dt.float32
F16 = mybir.dt.float16

_nc = None


def _build():
    nc = bacc.Bacc("TRN2", target_bir_lowering=False, debug=False, num_devices=NCORES)
    xt = nc.dram_tensor("xt", [128, 8, T], F16, kind="ExternalInput").ap()
    wq = nc.dram_tensor("wq", [128, 8 * H], F16, kind="ExternalInput").ap()
    wkv = nc.dram_tensor("wkv", [128, 8 * 2 * H], F16, kind="ExternalInput").ap()
    out = nc.dram_tensor("out", [H + 1, 4 * TB], F32, kind="ExternalOutput").ap()

    PEX = mybir.EngineType.PE
    DVE = mybir.EngineType.DVE

    with tile.TileContext(nc) as tc:
        pid = nc.partition_id(engines=[PEX, DVE])
        s = pid % 2
        sn = (pid + 1) % 2
        with tc.tile_pool(name="persist", bufs=1) as persist, \
             tc.tile_pool(name="work", bufs=1) as work, \
             tc.tile_pool(name="pp", bufs=1, space="PSUM") as pp:
            junk = persist.tile([128, TB], F16)   # warmup moving operand
            nc.gpsimd.memset(junk, 0.0)
            ident = persist.tile([128, 128], F16)
            make_identity(nc, ident)
            wq_sb = persist.tile([128, 8 * H], F16)
            wkv_sb = persist.tile([128, 8 * 2 * H], F16)
            nc.scalar.dma_start(out=wq_sb, in_=wq)
            nc.scalar.dma_start(out=wkv_sb, in_=wkv)

            # masks: [zeros, d0, d1, d2, d3, ones] built on GpSimd
            masks = persist.tile([128, 6 * TB], F16)
            nc.gpsimd.memset(masks[:, TB:], 1.0)
            nc.gpsimd.memset(masks[:, 0:TB], 0.0)
            for j in range(4):
                nc.gpsimd.affine_select(
                    out=masks[:, (1 + j) * TB:(2 + j) * TB],
                    in_=masks[:, (1 + j) * TB:(2 + j) * TB],
                    pattern=[[1, TB]],
                    compare_op=mybir.AluOpType.is_ge,
                    fill=0.0,
                    base=-128 * j,
                    channel_multiplier=-1,
                )

            xbig = persist.tile([128, 8, T], F16)
            QT = persist.tile([64, 4 * TB], F16)     # slot-compact Q^T
            KT = persist.tile([64, T], F16)
            V = persist.tile([128, NKVT, H + 1], F16)
            nc.gpsimd.memset(V[:, :, H], 1.0)        # row-sum ones column

            # x pieces (2 t-blocks each), stream order {6,7},{0,1},{2,3},{4,5}
            for p0 in (3072, 0, 1024, 2048):
                nc.sync.dma_start(
                    out=xbig[:, :, p0:p0 + 1024], in_=xt[:, :, p0:p0 + 1024]
                )

            def warm(n, cols=256):
                for _ in range(n):
                    pw = pp.tile([128, 2 * TB], F32, name="pw", tag="ps", bufs=2)
                    nc.tensor.matmul(pw[:, 0:cols], ident, junk[:, 0:cols],
                                     start=True, stop=True)

            def emit_qproj(slot, off):
                psq = pp.tile([128, TB], F32, name="psq", tag="pj", bufs=2)
                for c in range(8):
                    nc.tensor.matmul(
                        psq[0:64, :],
                        wq_sb[:, c * H:(c + 1) * H],
                        xbig[:, c, bass.ds(off, TB)],
                        start=(c == 0),
                        stop=(c == 7),
                    )
                nc.vector.tensor_copy(QT[:, slot * TB:(slot + 1) * TB], psq[0:64, :])

            def emit_kvproj_pair(tb0):
                tbs = (tb0, tb0 + 1)
                pv = []
                for tb in tbs:
                    pv.append(pp.tile([128, TB], F32, name=f"pvk{tb}", tag="pj",
                                      bufs=2))
                for c in range(8):
                    for i, tb in enumerate(tbs):
                        nc.tensor.matmul(
                            pv[i],
                            wkv_sb[:, c * 128:(c + 1) * 128],
                            xbig[:, c, tb * TB:(tb + 1) * TB],
                            start=(c == 0),
                            stop=(c == 7),
                        )
                for i, tb in enumerate(tbs):
                    nc.vector.tensor_copy(KT[:, tb * TB:(tb + 1) * TB], pv[i][0:64, :])
                    vt = work.tile([64, TB], F16, name="vt", tag="vt", bufs=2)
                    nc.vector.tensor_copy(vt, pv[i][64:128, :])
                    psv = pp.tile([128, TB], F16, name="psv", tag="pj", bufs=2)
                    for j in range(4):
                        nc.tensor.transpose(
                            psv[:, j * 64:(j + 1) * 64],
                            vt[:, j * 128:(j + 1) * 128],
                            ident[0:64, 0:64],
                        )
                    nc.vector.tensor_copy(V[:, 4 * tb:4 * tb + 4, 0:H], psv[:, 0:256])

            def mask_off(slot, r):
                # table [zeros,d0..d3,ones]; offsets affine in s, nonneg coeffs
                if slot < 2:
                    if r < 4:   # s=0: diag r, s=1: ones
                        return TB * (1 + r) + s * (TB * (4 - r))
                    else:       # s=0: zeros, s=1: diag r-4
                        return s * (TB * (r - 3))
                else:
                    assert r >= 4   # s=0: diag r-4 (TB*(r-3)), s=1: zeros (0)
                    return sn * (TB * (r - 3))

            def mask_off23_low(r):
                # slots 2,3 / r<4: s=0 -> ones (5*TB), s=1 -> diag r ((1+r)*TB)
                return nc.s_assert_within(
                    sn * (TB * 5) + s * (TB * (1 + r)), None, TB * 5,
                    skip_runtime_assert=True,
                )

            st = {}

            def emit_attn(slot, ks, pad=False):
                cnt = SLOT_TILES[slot]
                first = (ks[0] == (24 if slot == 3 else 0))
                last_k = 23 if slot == 3 else cnt - 1
                if first:
                    st[slot] = {
                        "po": pp.tile([H + 1, TB], F32, name=f"po{slot}",
                                      tag=("po_b" if slot == 3 else "po_a"),
                                      bufs=1),
                        "first": True,
                    }
                sd = st[slot]
                for idx in range(0, len(ks), 2):
                    kpair = ks[idx:idx + 2]
                    ps = pp.tile([128, 2 * TB], F32, name="ps", tag="ps", bufs=2)
                    for h, k in enumerate(kpair):
                        nc.tensor.matmul(
                            ps[:, h * TB:(h + 1) * TB],
                            KT[:, k * 128:(k + 1) * 128],
                            QT[:, slot * TB:(slot + 1) * TB],
                            start=True,
                            stop=True,
                        )
                    pt = work.tile([128, 2 * TB], F16, name="pt", tag="pt", bufs=4)
                    nc.scalar.activation(
                        pt, ps, mybir.ActivationFunctionType.Exp, scale=0.125
                    )
                    for h, k in enumerate(kpair):
                        r = k - (cnt - 8)
                        if r >= 0:
                            if slot >= 2 and r < 4:
                                off = mask_off23_low(r)
                            else:
                                off = mask_off(slot, r)
                            nc.vector.tensor_mul(
                                pt[:, h * TB:(h + 1) * TB],
                                pt[:, h * TB:(h + 1) * TB],
                                masks[:, bass.ds(off, TB)],
                            )
                    for h, k in enumerate(kpair):
                        nc.tensor.matmul(
                            sd["po"],
                            V[:, k, :],
                            pt[:, h * TB:(h + 1) * TB],
                            start=sd["first"],
                            stop=(k == last_k),
                        )
                        sd["first"] = False
                    if pad:
                        # keep the PE dense through the ACT-paced drain so the
                        # HAM clock gate stays at 8/8
                        warm(1)
                if ks[-1] == last_k:
                    ot = work.tile([H + 1, TB], F32, name="ot", tag="ot", bufs=2)
                    nc.vector.tensor_copy(ot, sd["po"])
                    nc.sync.dma_start(
                        out=out[:, slot * TB:(slot + 1) * TB], in_=ot
                    )

            # ---- emission schedule ----
            warm(36)
            emit_qproj(3, 3072 + sn * TB)      # block 7-s from piece {6,7}
            emit_kvproj_pair(6)                # kv tiles 24-31
            emit_attn(3, list(range(24, 32)))  # slot3 tail tiles first
            emit_kvproj_pair(0)
            emit_qproj(0, s * TB)              # block s
            emit_attn(0, list(range(0, 8)))
            emit_attn(3, list(range(0, 8)))
            emit_kvproj_pair(2)
            emit_qproj(1, 1024 + s * TB)       # block 2+s
            emit_attn(1, list(range(0, 16)))
            emit_attn(3, list(range(8, 16)))
            emit_kvproj_pair(4)
            emit_qproj(2, 2048 + sn * TB)      # block 5-s
            emit_attn(2, list(range(0, 24)), pad=True)
            emit_attn(3, list(range(16, 24)), pad=True)

    nc.compile()
    return nc


def get_nc():
    global _nc
    if _nc is None:
        _nc = _build()
    return _nc


def _pack_w(wt):
    # [C, M] (= W.T) -> [128, 8*M]: partition p, free c*M+m = wt[c*128+p, m]
    M = wt.shape[1]
    return np.ascontiguousarray(
        wt.reshape(8, 128, M).transpose(1, 0, 2).reshape(128, 8 * M)
    )


def make_inputs(x, Wq, Wk, Wv):
    x = np.asarray(x, dtype=np.float32).astype(np.float16)
    wq_in = _pack_w(np.asarray(Wq, np.float32).T.astype(np.float16))
    wkv_in = _pack_w(
        np.concatenate(
            [np.asarray(Wk, np.float32).T, np.asarray(Wv, np.float32).T], axis=1
        ).astype(np.float16)
    )
    in_maps = []
    for core in range(NCORES):
        b = core // 2
        # [T, C] -> [C, T] -> [8, 128, T] -> [128, 8, T]
        xp = np.ascontiguousarray(
            x[b].T.reshape(8, 128, T).transpose(1, 0, 2)
        )
        in_maps.append({"xt": xp, "wq": wq_in, "wkv": wkv_in})
    return in_maps


def gather_output(results):
    """results: per-core {"out": [65, 2048] f32} -> full [B, T, H] f32."""
    O = np.empty((B, T, H), np.float32)
    for core in range(NCORES):
        b, sv = core // 2, core % 2
        o = results[core]["out"].astype(np.float64)
        blocks = [sv, 2 + sv, 5 - sv, 7 - sv]
        for slot, g in enumerate(blocks):
            ot = o[0:64, slot * TB:(slot + 1) * TB]
            l = o[64, slot * TB:(slot + 1) * TB]
            O[b, g * TB:(g + 1) * TB] = (ot / l).T.astype(np.float32)
    return O


def kernel(x, Wq, Wk, Wv):
    nc = get_nc()
    in_maps = make_inputs(x, Wq, Wk, Wv)
    res = run_bass_kernel_spmd(nc, in_maps, list(range(NCORES)))
    return gather_output(res.results)
